# revision 2
# baseline (speedup 1.0000x reference)
"""BiLevelRoutingAttention Trainium2 kernel (8-core SPMD).

Sharding: core r handles batch b = r//4 and windows w in [ (r%4)*8, (r%4)*8+8 ).
Routing (region top-k) is computed on host via linearity of the mean:
    q_region = mean_{t,s}(xw) @ Wq + bq  (exact up to fp reassociation).
Spikes are binary -> all attention arithmetic is exact integer math in fp16/fp32.
Cross-window kv sums need R (per-region k^T v) from sibling cores -> AllGather
over the 4-core group of each batch.
"""

import numpy as np
import ml_dtypes

# ---- problem constants (hardcoded per contract) ----
T, B, Lt, Lh, Lw, C = 4, 2, 8, 32, 32, 256
WT, WH, WW = 2, 4, 4
W = WT * WH * WW            # 32 windows
GT, GH, GW = Lt // WT, Lh // WH, Lw // WW
S = GT * GH * GW            # 256 tokens per window
H, D = 8, C // 8            # 8 heads, 32 dim
TOPK = 8
SCALE = float(D) ** -0.5
NCORES = 8
NW = 8                      # windows per core
NTOK = T * S                # 1024 token-instances per window

MM_DT_NAME = "float32"      # matmul dtype for qkv/proj ("float32" or "float32r")

_prog_cache = {}


def _split_sync_waits(nc, mybir, maxw=1):
    """walrus in this container rejects >1 sync wait per instruction; split
    excess waits onto NoOp instructions inserted just before."""
    for bb in nc.main_func.blocks:
        new_list = []
        for ins in bb.instructions:
            si = ins.sync_info
            waits = list(si.on_wait) if si is not None and si.on_wait else []
            if len(waits) > maxw:
                extra = waits[:-maxw]
                keep = waits[-maxw:]
                idx = 0
                while extra:
                    chunk, extra = extra[:maxw], extra[maxw:]
                    nop = mybir.InstNoOp(name=f"{ins.name}-wsplit{idx}", ins=[], outs=[])
                    nop.engine = ins.engine
                    nop.sync_info = mybir.SyncInfo(on_wait=chunk, on_update=[])
                    new_list.append(nop)
                    idx += 1
                ins.sync_info = mybir.SyncInfo(
                    on_wait=keep,
                    on_update=list(si.on_update) if si.on_update else [],
                )
            new_list.append(ins)
        bb.instructions[:] = new_list


def _build_program(routing_idx):
    """routing_idx: [B, W, TOPK] int array (host-computed). Returns nc."""
    import concourse.bass as bass
    import concourse.mybir as mybir
    import concourse.tile as tile

    f32 = mybir.dt.float32
    f16 = mybir.dt.float16
    bf16 = mybir.dt.bfloat16
    mm_dt = getattr(mybir.dt, MM_DT_NAME)
    ALU = mybir.AluOpType
    ACT = mybir.ActivationFunctionType

    nc = bass.Bass(num_devices=NCORES)

    # ---- I/O ----
    x_in = nc.dram_tensor("x_in", [NW, 2, 128, NTOK], mm_dt, kind="ExternalInput")
    wkv_in = nc.dram_tensor("wkv_in", [2, 128, 512], mm_dt, kind="ExternalInput")
    bkv_hi_in = nc.dram_tensor("bkv_hi_in", [1, 512], bf16, kind="ExternalInput")
    bkv_lo_in = nc.dram_tensor("bkv_lo_in", [1, 512], bf16, kind="ExternalInput")
    wq_in = nc.dram_tensor("wq_in", [2, 128, 256], mm_dt, kind="ExternalInput")
    bq_in = nc.dram_tensor("bq_in", [2, 128, 1], f32, kind="ExternalInput")
    wp_in = nc.dram_tensor("wp_in", [2, 128, 256], mm_dt, kind="ExternalInput")
    bp_in = nc.dram_tensor("bp_in", [2, 128, 1], f32, kind="ExternalInput")
    out_d = nc.dram_tensor("out_d", [NW, 2, 128, NTOK], f32, kind="ExternalOutput")

    with tile.TileContext(nc) as tc:
        with (
            tc.tile_pool(name="const", bufs=1) as constp,
            tc.tile_pool(name="xin", bufs=2) as xin_p,
            tc.tile_pool(name="xkv", bufs=2) as xkv_p,
            tc.tile_pool(name="xq", bufs=2) as xq_p,
            tc.tile_pool(name="skv", bufs=2) as skv_p,
            tc.tile_pool(name="state", bufs=2) as st_p,
            tc.tile_pool(name="persist", bufs=1) as pers_p,
            tc.tile_pool(name="attn", bufs=2) as attn_p,
            tc.tile_pool(name="outs", bufs=2) as out_p,
            tc.tile_pool(name="psmm", bufs=3, space="PSUM") as ps_mm,
            tc.tile_pool(name="psr", bufs=2, space="PSUM") as ps_r,
            tc.tile_pool(name="psat", bufs=2, space="PSUM") as ps_at,
            tc.tile_pool(name="dram", bufs=1, space="DRAM") as dram_p,
        ):
            # ---- constants / weights ----
            wkv_sb = constp.tile([128, 2 * 512], mm_dt)
            for kc in range(2):
                nc.sync.dma_start(wkv_sb[:, kc * 512:(kc + 1) * 512], wkv_in[kc])
            wq_sb = constp.tile([128, 2 * 256], mm_dt)
            for kc in range(2):
                nc.sync.dma_start(wq_sb[:, kc * 256:(kc + 1) * 256], wq_in[kc])
            wp_sb = constp.tile([128, 2 * 256], mm_dt)
            for kc in range(2):
                nc.sync.dma_start(wp_sb[:, kc * 256:(kc + 1) * 256], wp_in[kc])
            bq_sb = constp.tile([128, 2], f32)
            bp_sb = constp.tile([128, 2], f32)
            for ftc in range(2):
                nc.sync.dma_start(bq_sb[:, ftc:ftc + 1], bq_in[ftc])
                nc.sync.dma_start(bp_sb[:, ftc:ftc + 1], bp_in[ftc])
            bkv_hi = constp.tile([1, 512], bf16)
            bkv_lo = constp.tile([1, 512], bf16)
            nc.sync.dma_start(bkv_hi[:], bkv_hi_in[:])
            nc.sync.dma_start(bkv_lo[:], bkv_lo_in[:])
            ones_row = constp.tile([1, 128], bf16)
            nc.vector.memset(ones_row[:], 1.0)

            # persistent across phases
            sq_all = pers_p.tile([128, NW * 2048], f16)    # q spikes, feature-major
            r_loc = pers_p.tile([128, 2048], f16)          # local R, (slab,t,wi,e)
            r_all = pers_p.tile([128, 8192], f16)          # gathered R, (slab,t,j,e)
            kv_all = pers_p.tile([128, NW * 256], f16)     # kv per local w, (slab,t,e)

            # ================= phase 1: per-window qkv + LIF + R =================
            for wi in range(NW):
                x_sb = xin_p.tile([128, 2 * NTOK], mm_dt, tag="xsb")
                for kc in range(2):
                    nc.sync.dma_start(x_sb[:, kc * NTOK:(kc + 1) * NTOK], x_in[wi, kc])

                # ---- pass A: k,v (token-major) ----
                xkv = xkv_p.tile([128, 8 * 512], f32, tag="xkv")
                for st in range(8):  # st = t*2 + sh
                    ps = ps_mm.tile([128, 512], f32, tag="mm")
                    for kc in range(2):
                        nc.tensor.matmul(
                            ps[:],
                            lhsT=x_sb[:, kc * NTOK + st * 128: kc * NTOK + (st + 1) * 128],
                            rhs=wkv_sb[:, kc * 512:(kc + 1) * 512],
                            start=(kc == 0), stop=False,
                        )
                    nc.tensor.matmul(ps[:], lhsT=ones_row[:], rhs=bkv_hi[:],
                                     start=False, stop=False)
                    nc.tensor.matmul(ps[:], lhsT=ones_row[:], rhs=bkv_lo[:],
                                     start=False, stop=True)
                    nc.scalar.activation(xkv[:, st * 512:(st + 1) * 512], ps[:],
                                         ACT.Copy, bias=0.0, scale=1.0)

                # ---- pass B: q (feature-major) ----
                xq = xq_p.tile([128, 2 * NTOK], f32, tag="xq")
                for ftc in range(2):
                    for nch in range(2):
                        psq = ps_mm.tile([128, 512], f32, tag="mm")
                        for kc in range(2):
                            nc.tensor.matmul(
                                psq[:],
                                lhsT=wq_sb[:, kc * 256 + ftc * 128: kc * 256 + (ftc + 1) * 128],
                                rhs=x_sb[:, kc * NTOK + nch * 512: kc * NTOK + (nch + 1) * 512],
                                start=(kc == 0), stop=(kc == 1),
                            )
                        nc.scalar.activation(
                            xq[:, ftc * NTOK + nch * 512: ftc * NTOK + (nch + 1) * 512],
                            psq[:], ACT.Identity, bias=bq_sb[:, ftc:ftc + 1], scale=1.0)

                # ---- LIF on k,v (token-major; per token-half sh) ----
                skv = skv_p.tile([128, 8 * 512], f16, tag="skv")
                for sh in range(2):
                    vkv = st_p.tile([128, 512], f32, tag="vkv")
                    lt = st_p.tile([128, 512], f32, tag="ltkv")
                    for t in range(T):
                        X = xkv[:, (t * 2 + sh) * 512:(t * 2 + sh + 1) * 512]
                        if t > 0:
                            nc.vector.tensor_tensor(X, X, vkv[:], op=ALU.add)
                        nc.vector.tensor_scalar(
                            skv[:, (t * 2 + sh) * 512:(t * 2 + sh + 1) * 512],
                            X, 1.0, None, ALU.is_ge)
                        if t < T - 1:
                            nc.vector.tensor_scalar(lt[:], X, 1.0, 0.5, ALU.is_lt, ALU.mult)
                            nc.vector.tensor_tensor(vkv[:], X, lt[:], op=ALU.mult)

                # ---- LIF on q (feature-major; per feature-tile) ----
                for ftc in range(2):
                    vq = st_p.tile([128, 256], f32, tag="vq")
                    ltq = st_p.tile([128, 256], f32, tag="ltq")
                    for t in range(T):
                        X = xq[:, ftc * NTOK + t * 256: ftc * NTOK + (t + 1) * 256]
                        if t > 0:
                            nc.vector.tensor_tensor(X, X, vq[:], op=ALU.add)
                        nc.vector.tensor_scalar(
                            sq_all[:, wi * 2048 + ftc * NTOK + t * 256:
                                   wi * 2048 + ftc * NTOK + (t + 1) * 256],
                            X, 1.0, None, ALU.is_ge)
                        if t < T - 1:
                            nc.vector.tensor_scalar(ltq[:], X, 1.0, 0.5, ALU.is_lt, ALU.mult)
                            nc.vector.tensor_tensor(vq[:], X, ltq[:], op=ALU.mult)

                # ---- R = k^T v per (t, head): [d,e] blocks, col-tiled 4 heads ----
                for t in range(T):
                    for slab in range(2):
                        psr = ps_r.tile([128, 32], f32, tag="psr")
                        for hl in range(4):
                            h = slab * 4 + hl
                            for sh in range(2):
                                st = t * 2 + sh
                                nc.tensor.matmul(
                                    psr[32 * hl:32 * (hl + 1), :],
                                    lhsT=skv[:, st * 512 + h * 32: st * 512 + (h + 1) * 32],
                                    rhs=skv[:, st * 512 + 256 + h * 32: st * 512 + 256 + (h + 1) * 32],
                                    start=(sh == 0), stop=(sh == 1),
                                    tile_position=(0, 32 * hl),
                                )
                        nc.scalar.activation(
                            r_loc[:, ((slab * 4 + t) * 8 + wi) * 32:
                                  ((slab * 4 + t) * 8 + wi + 1) * 32],
                            psr[:], ACT.Copy, bias=0.0, scale=1.0)

            # ================= phase 2: exchange R, kv sums, attention, proj ======
            rb_in = dram_p.tile([128, 2048], f16)
            rb_out = dram_p.tile([4, 128, 2048], f16)
            nc.sync.dma_start(rb_in[:], r_loc[:])
            nc.gpsimd.collective_compute(
                "AllGather",
                mybir.AluOpType.bypass,
                replica_groups=[[0, 1, 2, 3], [4, 5, 6, 7]],
                ins=[rb_in[:].opt()],
                outs=[rb_out[:].opt()],
            )
            # r_all free layout: (slab2, t4, j32, e32)
            r_all_v = r_all[:].rearrange("p (a t j e) -> p a t j e", a=2, t=4, j=32, e=32)
            for rk in range(4):
                src = rb_out[rk].rearrange("p (a t w e) -> p a t w e", a=2, t=4, w=8, e=32)
                nc.sync.dma_start(r_all_v[:, :, :, rk * 8:(rk + 1) * 8, :], src)

            # kv sums: routed gather baked per core, guarded by If on core id
            pid = nc.partition_id()
            for r in range(NCORES):
                b_of = r // 4
                wg = r % 4
                with tc.If(pid == r):
                    for wl in range(NW):
                        wglob = wg * 8 + wl
                        idxs = [int(j) for j in routing_idx[b_of, wglob]]
                        dst = kv_all[:, wl * 256:(wl + 1) * 256]
                        src0 = r_all_v[:, :, :, idxs[0], :]
                        nc.vector.tensor_copy(dst, src0)
                        for j in idxs[1:]:
                            nc.vector.tensor_tensor(
                                dst, dst, r_all_v[:, :, :, j, :], op=ALU.add)

            # attention out + proj per window
            for wi in range(NW):
                attn = attn_p.tile([128, 2 * NTOK], f32, tag="attn")
                for t in range(T):
                    for slab in range(2):
                        psa = ps_at.tile([128, 256], f32, tag="psa")
                        for hl in range(4):
                            nc.tensor.matmul(
                                psa[32 * hl:32 * (hl + 1), :],
                                lhsT=kv_all[32 * hl:32 * (hl + 1),
                                            wi * 256 + (slab * 4 + t) * 32:
                                            wi * 256 + (slab * 4 + t + 1) * 32],
                                rhs=sq_all[32 * hl:32 * (hl + 1),
                                           wi * 2048 + slab * NTOK + t * 256:
                                           wi * 2048 + slab * NTOK + (t + 1) * 256],
                                start=True, stop=True,
                                tile_position=(32 * hl, 32 * hl),
                            )
                        nc.scalar.activation(
                            attn[:, slab * NTOK + t * 256: slab * NTOK + (t + 1) * 256],
                            psa[:], ACT.Copy, bias=0.0, scale=SCALE)

                outsb = out_p.tile([128, 2 * NTOK], f32, tag="outsb")
                for cft in range(2):
                    for nch in range(2):
                        psp = ps_mm.tile([128, 512], f32, tag="mm")
                        for kc in range(2):
                            nc.tensor.matmul(
                                psp[:],
                                lhsT=wp_sb[:, kc * 256 + cft * 128: kc * 256 + (cft + 1) * 128],
                                rhs=attn[:, kc * NTOK + nch * 512: kc * NTOK + (nch + 1) * 512],
                                start=(kc == 0), stop=(kc == 1),
                            )
                        nc.scalar.activation(
                            outsb[:, cft * NTOK + nch * 512: cft * NTOK + (nch + 1) * 512],
                            psp[:], ACT.Identity, bias=bp_sb[:, cft:cft + 1], scale=1.0)
                for cft in range(2):
                    nc.sync.dma_start(out_d[wi, cft], outsb[:, cft * NTOK:(cft + 1) * NTOK])

    _split_sync_waits(nc, mybir, maxw=1)
    return nc


def _host_prepost(x, w_qkv, b_qkv):
    """Window partition, routing, per-core input arrays."""
    xw = x.reshape(T, B, WT, GT, WH, GH, WW, GW, C) \
          .transpose(0, 1, 2, 4, 6, 3, 5, 7, 8).reshape(T, B, W, S, C)
    xbar = xw.mean(axis=(0, 3))                      # [B, W, C]
    q_reg = xbar @ w_qkv[:, :C] + b_qkv[:C]
    k_reg = xbar @ w_qkv[:, C:2 * C] + b_qkv[C:2 * C]
    a_r = np.einsum('bwc,bvc->bwv', q_reg, k_reg)
    routing_idx = np.argsort(-a_r, axis=-1)[:, :, :TOPK]   # [B, W, TOPK]
    return xw, routing_idx


def kernel(x, w_qkv, b_qkv, w_proj, b_proj):
    x = np.ascontiguousarray(np.asarray(x, dtype=np.float32))
    w_qkv = np.asarray(w_qkv, dtype=np.float32)
    b_qkv = np.asarray(b_qkv, dtype=np.float32)
    w_proj = np.asarray(w_proj, dtype=np.float32)
    b_proj = np.asarray(b_proj, dtype=np.float32)

    xw, routing_idx = _host_prepost(x, w_qkv, b_qkv)

    key = routing_idx.tobytes()
    if key not in _prog_cache:
        _prog_cache.clear()
        _prog_cache[key] = _build_program(routing_idx)
    nc = _prog_cache[key]

    # weights (shared across cores)
    wkv = (0.5 * w_qkv[:, C:]).reshape(2, 128, 512).astype(np.float32)
    bkv_half = (0.5 * b_qkv[C:]).astype(np.float32)
    bkv_hi = bkv_half.astype(ml_dtypes.bfloat16)
    bkv_lo = (bkv_half - bkv_hi.astype(np.float32)).astype(ml_dtypes.bfloat16)
    wq = (0.5 * w_qkv[:, :C]).reshape(2, 128, 256).astype(np.float32)
    bq = (0.5 * b_qkv[:C]).reshape(2, 128, 1).astype(np.float32)
    wp = w_proj.reshape(2, 128, 256).astype(np.float32)
    bp = b_proj.reshape(2, 128, 1).astype(np.float32)

    in_maps = []
    for r in range(NCORES):
        b_of, wg = r // 4, r % 4
        xwc = xw[:, b_of, wg * 8:(wg + 1) * 8]              # [T, 8, S, C]
        xl = np.ascontiguousarray(
            xwc.transpose(1, 3, 0, 2).reshape(NW, 2, 128, NTOK))
        in_maps.append({
            "x_in": xl,
            "wkv_in": wkv, "bkv_hi_in": bkv_hi.reshape(1, 512),
            "bkv_lo_in": bkv_lo.reshape(1, 512),
            "wq_in": wq, "bq_in": bq, "wp_in": wp, "bp_in": bp,
        })

    from concourse.bass_utils import run_bass_kernel_spmd
    res = run_bass_kernel_spmd(nc, in_maps, core_ids=list(range(NCORES)))

    # assemble output
    yw = np.empty((T, B, W, S, C), dtype=np.float32)
    for r in range(NCORES):
        b_of, wg = r // 4, r % 4
        o = res.results[r]["out_d"]                          # [NW, 2, 128, NTOK]
        o = o.reshape(NW, 2, 128, T, S).transpose(0, 3, 4, 1, 2).reshape(NW, T, S, C)
        for wl in range(NW):
            yw[:, b_of, wg * 8 + wl] = o[wl]

    y = yw.reshape(T, B, WT, WH, WW, GT, GH, GW, C) \
          .transpose(0, 1, 2, 5, 3, 6, 4, 7, 8).reshape(T, B, Lt, Lh, Lw, C)
    return y
